# revision 19
# baseline (speedup 1.0000x reference)
"""NonLocalBlock Trainium2 Bass kernel.

Reference computation (per full batch):
  xf = x.reshape(B, C, N)                      # N = S*S = 4096, C = 512
  v  = Wv @ xf  (per sample) -> v[m, o]        # [N, C]
  q  = Wq @ xf -> q[i, n]                      # [inter, N]
  k  = Wk @ xf -> k[i, m]                      # [inter, N]
  rel[n, m] = softmax_m( (q.T @ k)[n, m] / dist[n, m] )
  y[n, o] = rel @ v
  u = Wu @ y.T                                 # [C, N]
  out = relu(BN(u) + x)     (BN over batch+spatial, training stats)

Sharding: data-parallel over B across 8 cores (one sample per core).
BN statistics are combined with a tiny (128x8 f32) AllReduce.

Device-side design:
  - All matmuls on PE. Projections + QK use fp32 data issued as
    float32r (full-rate). softmax weights and the big rel@v matmul run
    in bf16 (all-positive weights; sub-1% error after normalization).
  - 1/dist is 2-level block-Toeplitz: rel row n needs the SxS window of
    a (2S-1)^2 table g at offset (S-1-i_n, S-1-j_n), and g is symmetric
    under axis reversal. Per-partition pre-shifted copies of g
    (host-prepared) let the DVE multiply read each partition's window
    in place via a strided AP - zero materialization traffic.
  - softmax skips max-subtraction (max |logit| ~ 69 << 88 so exp() fits
    fp32/bf16 range; all terms positive, no cancellation). exp() runs
    on ACT with accum_out producing the row-sum in the same pass; the
    normalization is folded into the PSUM->SBUF scale of y.
  - rel row-tiles are transposed 128x128 on PE (bf16, 1 cyc/row) so
    rel@v can contract over m with lhsT in SBUF.
"""

import os
import sys

import numpy as np

if "/opt/trn_rl_repo" not in sys.path:
    sys.path.insert(0, "/opt/trn_rl_repo")

import ml_dtypes

import concourse.bass as bass  # noqa: F401  (AP helpers)
import concourse.mybir as mybir
import concourse.tile as tile
from concourse import bacc
from concourse.bass_utils import run_bass_kernel_spmd

F32 = mybir.dt.float32
F32R = mybir.dt.float32r
BF16 = mybir.dt.bfloat16
AF = mybir.ActivationFunctionType
ALU = mybir.AluOpType

B = 8
C = 512
INTER = 128
BN_EPS = 1e-5


def _r(ap):
    """fp32 AP -> float32r view for full-rate PE matmuls."""
    return ap.bitcast(F32R)


def build_nc(S=64, n_cores=8):
    """Build + compile the per-core Bass program (SPMD, one sample/core)."""
    N = S * S
    GROWS = 2 * S - 1          # g table rows
    PADW = 2 * S               # padded g row width
    RPT = 128 // S             # grid rows per 128-row tile
    NT = N // 128              # n row-tiles
    MC = N // 128              # m chunks of 128
    NBLK = min(512, N)         # QK / rel free-dim block
    NMB = max(1, N // NBLK)    # m blocks per row-tile
    IMB = NBLK // S            # grid rows of m per block
    GW = GROWS * PADW + (RPT - 1) * PADW + S  # per-partition g buffer width
    UG = min(4, NT)            # row-tiles per u group (512 cols of n)
    TG = min(4, MC)            # rel-transpose group size
    OT = C // 128              # output-channel tiles
    KC = C // 128              # K chunks over C
    NH = 16 if N > 2048 else 1  # phase-1 n slices (SBUF pressure)
    NTOT = float(B * N)        # BN divisor

    nc = bacc.Bacc(
        "TRN2", target_bir_lowering=False, debug=False, num_devices=n_cores
    )

    x_in = nc.dram_tensor("x", [C, N], F32R, kind="ExternalInput").ap()
    wvt = nc.dram_tensor("wvt", [C, C], F32R, kind="ExternalInput").ap()
    wqt = nc.dram_tensor("wqt", [C, INTER], F32R, kind="ExternalInput").ap()
    wkt = nc.dram_tensor("wkt", [C, INTER], F32R, kind="ExternalInput").ap()
    wut = nc.dram_tensor("wut", [C, C], BF16, kind="ExternalInput").ap()
    gsrep_in = nc.dram_tensor("gsrep", [128, GW], F32, kind="ExternalInput").ap()
    ident_in = nc.dram_tensor("ident", [128, 128], BF16, kind="ExternalInput").ap()
    gamma_in = nc.dram_tensor("gamma", [128, OT], F32, kind="ExternalInput").ap()
    beta_in = nc.dram_tensor("beta", [128, OT], F32, kind="ExternalInput").ap()
    out_ext = nc.dram_tensor("out", [C, N], F32, kind="ExternalOutput").ap()

    with tile.TileContext(nc) as tc:
        with (
            tc.tile_pool(name="singles", bufs=1) as singles,
            tc.tile_pool(name="xc", bufs=KC) as xpool,
            tc.tile_pool(name="work", bufs=3) as work,
            tc.tile_pool(name="uwork", bufs=2) as uwork,
            tc.tile_pool(name="relb", bufs=(N // min(1024, N)) + 1) as relbp,
            tc.tile_pool(name="rT", bufs=3) as rTp,
            tc.tile_pool(name="yT", bufs=KC + 1) as yTp,
            tc.tile_pool(name="ysb", bufs=UG + 1) as ypool,
            tc.tile_pool(name="small", bufs=4) as small,
            tc.tile_pool(name="qk_ps", bufs=2, space="PSUM") as qkps,
            tc.tile_pool(name="tp_ps", bufs=2, space="PSUM") as tpps,
            tc.tile_pool(name="y_ps", bufs=2, space="PSUM") as yps,
            tc.tile_pool(name="dram", bufs=1, space="DRAM") as dram,
        ):
            # ---------- constants / weights ----------
            wvt_sb = singles.tile([128, KC, C], F32R, tag="wvt")
            nc.sync.dma_start(
                out=wvt_sb, in_=wvt.rearrange("(kc p) o -> p kc o", p=128)
            )
            wqt_sb = singles.tile([128, KC, INTER], F32R, tag="wqt")
            nc.sync.dma_start(
                out=wqt_sb, in_=wqt.rearrange("(kc p) i -> p kc i", p=128)
            )
            wkt_sb = singles.tile([128, KC, INTER], F32R, tag="wkt")
            nc.sync.dma_start(
                out=wkt_sb, in_=wkt.rearrange("(kc p) i -> p kc i", p=128)
            )
            wut_sb = singles.tile([128, KC, C], BF16, tag="wut")
            nc.sync.dma_start(
                out=wut_sb, in_=wut.rearrange("(kc p) o -> p kc o", p=128)
            )
            ident = singles.tile([128, 128], BF16, tag="ident")
            nc.sync.dma_start(out=ident, in_=ident_in)
            gamma_sb = singles.tile([128, OT], F32, tag="gamma")
            nc.sync.dma_start(out=gamma_sb, in_=gamma_in)
            beta_sb = singles.tile([128, OT], F32, tag="beta")
            nc.sync.dma_start(out=beta_sb, in_=beta_in)
            eps_sb = singles.tile([128, 1], F32, tag="eps")
            nc.vector.memset(eps_sb, BN_EPS)

            q_sb = singles.tile([128, N], F32R, tag="q")
            k_sb = singles.tile([128, N], F32R, tag="k")
            v_sb = singles.tile([128, MC, 512], BF16, tag="v")
            gsrep = singles.tile([128, GW], F32, tag="gsrep")
            nc.sync.dma_start(out=gsrep, in_=gsrep_in)

            sum_acc = singles.tile([128, OT], F32, tag="sum_acc")
            ssq_acc = singles.tile([128, OT], F32, tag="ssq_acc")
            nc.vector.memset(sum_acc, 0.0)
            nc.vector.memset(ssq_acc, 0.0)

            u_dram = dram.tile([C, N], F32)

            # ---------- phase 1: projections (q, k, v) ----------
            NHW = N // NH
            for nh in range(NH):
                nlo = nh * NHW
                x_sb = [
                    xpool.tile([128, NHW], F32R, tag="xc", name=f"xc{nh}_{i}")
                    for i in range(KC)
                ]
                for cc in range(KC):
                    nc.sync.dma_start(
                        out=x_sb[cc],
                        in_=x_in[cc * 128 : (cc + 1) * 128, nlo : nlo + NHW],
                    )
                PB = min(512, NHW)
                for nb in range(NHW // PB):
                    ps_q = qkps.tile([128, 512], F32, tag="qk")
                    ps_k = qkps.tile([128, 512], F32, tag="qk")
                    for cc in range(KC):
                        nc.tensor.matmul(
                            ps_q[:INTER, :PB],
                            wqt_sb[:, cc, :],
                            x_sb[cc][:, nb * PB : (nb + 1) * PB],
                            start=(cc == 0),
                            stop=(cc == KC - 1),
                        )
                    for cc in range(KC):
                        nc.tensor.matmul(
                            ps_k[:INTER, :PB],
                            wkt_sb[:, cc, :],
                            x_sb[cc][:, nb * PB : (nb + 1) * PB],
                            start=(cc == 0),
                            stop=(cc == KC - 1),
                        )
                    col = nlo + nb * PB
                    nc.vector.tensor_copy(
                        q_sb[:INTER, col : col + PB], ps_q[:INTER, :PB]
                    )
                    nc.vector.tensor_copy(
                        k_sb[:INTER, col : col + PB], ps_k[:INTER, :PB]
                    )
                for mt in range(NHW // 128):
                    mtg = nlo // 128 + mt
                    ps_v = qkps.tile([128, 512], F32, tag="qk")
                    for cc in range(KC):
                        nc.tensor.matmul(
                            ps_v,
                            x_sb[cc][:, mt * 128 : (mt + 1) * 128],
                            wvt_sb[:, cc, :],
                            start=(cc == 0),
                            stop=(cc == KC - 1),
                        )
                    nc.vector.tensor_copy(v_sb[:, mtg, :], ps_v)

            # ---------- phase 2: attention + u ----------
            QKB = min(1024, N)     # softmax block width (2 PSUM banks)
            NQB = N // QKB         # blocks per row-tile
            IM2 = QKB // S         # grid rows of m per block
            y_list = []
            for t in range(NT):
                i0 = t * RPT
                # per-partition window offset into the shifted g table
                off = (S - 1 - i0) * PADW + (S - 1)
                relb = []
                den8 = small.tile([128, NQB], F32, tag="den8")
                for mb in range(NQB):
                    ps = qkps.tile([128, QKB], F32, tag="qk")
                    for hf in range(QKB // NBLK):
                        nc.tensor.matmul(
                            ps[:, hf * NBLK : (hf + 1) * NBLK],
                            q_sb[:INTER, t * 128 : (t + 1) * 128],
                            k_sb[
                                :INTER,
                                mb * QKB + hf * NBLK : mb * QKB + (hf + 1) * NBLK,
                            ],
                            start=True,
                            stop=True,
                        )
                    gwin = gsrep[
                        :, off + mb * IM2 * PADW : off + (mb + 1) * IM2 * PADW
                    ].rearrange("p (i j) -> p i j", j=PADW)[:, :, :S]
                    rf = work.tile([128, QKB], F32, tag="relf", bufs=2)
                    nc.vector.tensor_mul(
                        rf.rearrange("p (i j) -> p i j", j=S),
                        ps.rearrange("p (i j) -> p i j", j=S),
                        gwin,
                    )
                    rb = relbp.tile([128, QKB], BF16, tag="relb")
                    nc.scalar.activation(
                        out=rb,
                        in_=rf,
                        func=AF.Exp,
                        accum_out=den8[:, mb : mb + 1],
                    )
                    relb.append(rb)
                den = small.tile([128, 1], F32, tag="den")
                nc.vector.reduce_sum(den, den8, axis=mybir.AxisListType.X)
                rden = small.tile([128, 1], F32, tag="rden")
                nc.vector.reciprocal(rden, den)

                y_ps = yps.tile([128, 512], F32, tag="y")
                for tg in range(MC // TG):
                    tp = tpps.tile([128, 512], BF16, tag="tp")
                    for j in range(TG):
                        mc = tg * TG + j
                        blk = relb[mc * 128 // QKB]
                        co = (mc * 128) % QKB
                        nc.tensor.transpose(
                            tp[:, j * 128 : (j + 1) * 128],
                            blk[:, co : co + 128],
                            ident,
                        )
                    rT = rTp.tile([128, 512], BF16, tag="rT")
                    nc.vector.tensor_copy(
                        rT[:, : TG * 128], tp[:, : TG * 128]
                    )
                    for j in range(TG):
                        mc = tg * TG + j
                        nc.tensor.matmul(
                            y_ps,
                            rT[:, j * 128 : (j + 1) * 128],
                            v_sb[:, mc, :],
                            start=(mc == 0),
                            stop=(mc == MC - 1),
                        )
                y_sb = ypool.tile([128, 512], BF16, tag="y_sb")
                nc.vector.tensor_scalar(
                    out=y_sb, in0=y_ps, scalar1=rden, scalar2=None, op0=ALU.mult
                )
                y_list.append(y_sb)

                # ---------- u for each completed group of UG row-tiles ----------
                if t % UG == UG - 1:
                    g4 = t // UG
                    ys = y_list[g4 * UG : (g4 + 1) * UG]
                    un = UG * 128
                    yT = []
                    for c in range(OT):
                        tp2 = tpps.tile([128, 512], BF16, tag="tp")
                        for r2 in range(UG):
                            nc.tensor.transpose(
                                tp2[:, r2 * 128 : (r2 + 1) * 128],
                                ys[r2][:, c * 128 : (c + 1) * 128],
                                ident,
                            )
                        yTc = yTp.tile([128, 512], BF16, tag="yT")
                        nc.vector.tensor_copy(yTc[:, :un], tp2[:, :un])
                        yT.append(yTc)
                    for ot in range(OT):
                        u_ps = yps.tile([128, 512], F32, tag="y")
                        for cc2 in range(KC):
                            nc.tensor.matmul(
                                u_ps[:, :un],
                                wut_sb[:, cc2, ot * 128 : (ot + 1) * 128],
                                yT[cc2][:, :un],
                                start=(cc2 == 0),
                                stop=(cc2 == KC - 1),
                            )
                        u_sb = uwork.tile([128, 512], F32, tag="usb")
                        su = small.tile([128, 1], F32, tag="su")
                        nc.vector.tensor_scalar(
                            out=u_sb[:, :un],
                            in0=u_ps[:, :un],
                            scalar1=1.0,
                            scalar2=0.0,
                            op0=ALU.mult,
                            op1=ALU.add,
                            accum_out=su,
                        )
                        usq = uwork.tile([128, 512], F32, tag="usq", bufs=1)
                        ssq = small.tile([128, 1], F32, tag="ssq")
                        nc.scalar.activation(
                            out=usq[:, :un],
                            in_=u_ps[:, :un],
                            func=AF.Square,
                            accum_out=ssq,
                        )
                        nc.vector.tensor_add(
                            sum_acc[:, ot : ot + 1], sum_acc[:, ot : ot + 1], su
                        )
                        nc.vector.tensor_add(
                            ssq_acc[:, ot : ot + 1], ssq_acc[:, ot : ot + 1], ssq
                        )
                        nc.sync.dma_start(
                            out=u_dram[
                                ot * 128 : (ot + 1) * 128, g4 * un : (g4 + 1) * un
                            ],
                            in_=u_sb[:, :un],
                        )

            # ---------- phase 3: BN stats AllReduce ----------
            stats_sb = small.tile([128, 2 * OT], F32, tag="stats")
            nc.vector.tensor_copy(stats_sb[:, :OT], sum_acc)
            nc.vector.tensor_copy(stats_sb[:, OT:], ssq_acc)
            cc_in = dram.tile([128, 2 * OT], F32)
            cc_out = dram.tile([128, 2 * OT], F32)
            nc.sync.dma_start(out=cc_in, in_=stats_sb)
            nc.gpsimd.collective_compute(
                "AllReduce",
                ALU.add,
                replica_groups=[list(range(n_cores))],
                ins=[cc_in.opt()],
                outs=[cc_out.opt()],
            )
            gstats = small.tile([128, 2 * OT], F32, tag="gstats")
            nc.sync.dma_start(out=gstats, in_=cc_out)

            mean = small.tile([128, OT], F32, tag="mean")
            nc.vector.tensor_scalar(
                out=mean, in0=gstats[:, :OT], scalar1=1.0 / NTOT,
                scalar2=None, op0=ALU.mult,
            )
            ex2 = small.tile([128, OT], F32, tag="ex2")
            nc.vector.tensor_scalar(
                out=ex2, in0=gstats[:, OT:], scalar1=1.0 / NTOT,
                scalar2=None, op0=ALU.mult,
            )
            var = small.tile([128, OT], F32, tag="var")
            nc.vector.tensor_mul(var, mean, mean)
            nc.vector.tensor_sub(var, ex2, var)
            std = small.tile([128, OT], F32, tag="std")
            nc.scalar.activation(out=std, in_=var, func=AF.Sqrt, bias=eps_sb)
            rstd = small.tile([128, OT], F32, tag="rstd")
            nc.vector.reciprocal(rstd, std)
            scale = small.tile([128, OT], F32, tag="scale")
            nc.vector.tensor_mul(scale, gamma_sb, rstd)
            shift = small.tile([128, OT], F32, tag="shift")
            nc.vector.tensor_mul(shift, mean, scale)
            nc.vector.tensor_sub(shift, beta_sb, shift)

            # ---------- phase 4: normalize + residual + relu ----------
            FW = min(256, N)
            for ot in range(OT):
                for h in range(N // FW):
                    ut = work.tile([128, FW], F32, tag="f_u", bufs=2)
                    xt = work.tile([128, FW], F32, tag="f_x", bufs=2)
                    nc.sync.dma_start(
                        out=ut,
                        in_=u_dram[
                            ot * 128 : (ot + 1) * 128, h * FW : (h + 1) * FW
                        ],
                    )
                    nc.sync.dma_start(
                        out=xt,
                        in_=x_in[
                            ot * 128 : (ot + 1) * 128, h * FW : (h + 1) * FW
                        ].bitcast(F32),
                    )
                    nc.vector.tensor_scalar(
                        out=ut,
                        in0=ut,
                        scalar1=scale[:, ot : ot + 1],
                        scalar2=shift[:, ot : ot + 1],
                        op0=ALU.mult,
                        op1=ALU.add,
                    )
                    nc.vector.tensor_add(ut, ut, xt)
                    o_sb = work.tile([128, FW], F32, tag="f_o", bufs=2)
                    nc.scalar.activation(out=o_sb, in_=ut, func=AF.Relu)
                    nc.sync.dma_start(
                        out=out_ext[
                            ot * 128 : (ot + 1) * 128, h * FW : (h + 1) * FW
                        ],
                        in_=o_sb,
                    )

    nc.compile()
    return nc


def make_host_inputs(x, Wv, Wq, Wk, Wu, bn_gamma, bn_beta, S=64):
    """Prepare per-core input maps (host-side constant prep)."""
    N = S * S
    GROWS = 2 * S - 1
    PADW = 2 * S
    RPT = 128 // S
    GW = GROWS * PADW + (RPT - 1) * PADW + S
    OT = C // 128

    # 1/dist sliding-window table: g[p, q] = 1/(hypot(p-(S-1), q-(S-1))+1).
    # g is symmetric under axis reversal, so the same table serves all rows.
    dd = np.arange(GROWS, dtype=np.float32) - (S - 1)
    g = 1.0 / (np.sqrt(dd[:, None] ** 2 + dd[None, :] ** 2) + 1.0)
    gpad = np.zeros((GROWS, PADW), dtype=np.float32)
    gpad[:, :GROWS] = g
    gflat = gpad.reshape(-1)
    gsrep = np.zeros((128, GW), dtype=np.float32)
    for r in range(128):
        off = (r // S) * PADW + (r % S)
        gsrep[r, off : off + gflat.size] = gflat

    wvt = np.ascontiguousarray(np.asarray(Wv).T).astype(np.float32)
    wqt = np.ascontiguousarray(np.asarray(Wq).T).astype(np.float32)
    wkt = np.ascontiguousarray(np.asarray(Wk).T).astype(np.float32)
    wut = np.ascontiguousarray(np.asarray(Wu).T).astype(ml_dtypes.bfloat16)
    ident = np.eye(128, dtype=ml_dtypes.bfloat16)
    gam = np.ascontiguousarray(
        np.asarray(bn_gamma).astype(np.float32).reshape(OT, 128).T
    )
    bet = np.ascontiguousarray(
        np.asarray(bn_beta).astype(np.float32).reshape(OT, 128).T
    )

    shared = dict(
        wvt=wvt, wqt=wqt, wkt=wkt, wut=wut,
        gsrep=gsrep, ident=ident, gamma=gam, beta=bet,
    )
    in_maps = []
    for b in range(x.shape[0]):
        m = dict(shared)
        m["x"] = np.ascontiguousarray(
            np.asarray(x[b]).reshape(C, N)
        ).astype(np.float32)
        in_maps.append(m)
    return in_maps


def _ensure_profile_hook():
    """This image's antenv lacks axon_hooks; synthesize it and register
    the ctypes NTFF hook so run_bass_kernel_spmd(trace=True) works."""
    import types

    try:
        import antenv.axon_hooks  # noqa: F401

        return
    except ImportError:
        pass
    try:
        import antenv
    except ImportError:
        return
    mod = types.ModuleType("antenv.axon_hooks")
    holder = {"h": None}
    mod.set_axon_ntff_profile_hook = lambda h: holder.__setitem__("h", h)
    mod.get_axon_ntff_profile_hook = lambda: holder["h"]
    sys.modules["antenv.axon_hooks"] = mod
    antenv.axon_hooks = mod
    try:
        from trn_agent_boot.trn_boot import _ntff_profile_via_ctypes

        hook = _ntff_profile_via_ctypes("/opt/axon/libaxon_pjrt.so")
        if hook is not None:
            mod.set_axon_ntff_profile_hook(hook)
    except Exception:
        pass


_NC_CACHE = {}


def kernel(x, Wv, Wq, Wk, Wu, bn_gamma, bn_beta):
    x = np.asarray(x)
    S = x.shape[2]
    n_cores = x.shape[0]
    key = (S, n_cores)
    if key not in _NC_CACHE:
        _NC_CACHE[key] = build_nc(S=S, n_cores=n_cores)
    nc = _NC_CACHE[key]

    in_maps = make_host_inputs(x, Wv, Wq, Wk, Wu, bn_gamma, bn_beta, S=S)
    trace = bool(int(os.environ.get("KERNEL_TRACE", "0")))
    if trace:
        _ensure_profile_hook()
    res = run_bass_kernel_spmd(
        nc, in_maps, core_ids=list(range(n_cores)), trace=trace
    )
    if trace and res.exec_time_ns is not None:
        print(f"HW exec time: {res.exec_time_ns} ns")
        kernel.last_exec_time_ns = res.exec_time_ns
    out = np.stack([r["out"].reshape(C, S, S) for r in res.results])
    return out.astype(np.float32)


# revision 20
# speedup vs baseline: 1.1710x; 1.1710x over previous
"""NonLocalBlock Trainium2 Bass kernel.

Reference computation (per full batch):
  xf = x.reshape(B, C, N)                      # N = S*S = 4096, C = 512
  v  = Wv @ xf  (per sample) -> v[m, o]        # [N, C]
  q  = Wq @ xf -> q[i, n]                      # [inter, N]
  k  = Wk @ xf -> k[i, m]                      # [inter, N]
  rel[n, m] = softmax_m( (q.T @ k)[n, m] / dist[n, m] )
  y[n, o] = rel @ v
  u = Wu @ y.T                                 # [C, N]
  out = relu(BN(u) + x)     (BN over batch+spatial, training stats)

Sharding: data-parallel over B across 8 cores (one sample per core).
BN statistics are combined with a tiny (128x8 f32) AllReduce.

Device-side design:
  - All matmuls on PE. Projections + QK use fp32 data issued as
    float32r (full-rate). softmax weights and the big rel@v matmul run
    in bf16 (all-positive weights; sub-1% error after normalization).
  - 1/dist is 2-level block-Toeplitz: rel row n needs the SxS window of
    a (2S-1)^2 table g at offset (S-1-i_n, S-1-j_n), and g is symmetric
    under axis reversal. Per-partition pre-shifted copies of g
    (host-prepared) let the DVE multiply read each partition's window
    in place via a strided AP - zero materialization traffic.
  - softmax skips max-subtraction (max |logit| ~ 69 << 88 so exp() fits
    fp32/bf16 range; all terms positive, no cancellation). exp() runs
    on ACT with accum_out producing the row-sum in the same pass; the
    normalization is folded into the PSUM->SBUF scale of y.
  - rel row-tiles are transposed 128x128 on PE (bf16, 1 cyc/row) so
    rel@v can contract over m with lhsT in SBUF.
"""

import os
import sys

import numpy as np

if "/opt/trn_rl_repo" not in sys.path:
    sys.path.insert(0, "/opt/trn_rl_repo")

import ml_dtypes

import concourse.bass as bass  # noqa: F401  (AP helpers)
import concourse.mybir as mybir
import concourse.tile as tile
from concourse import bacc
from concourse.bass_utils import run_bass_kernel_spmd

F32 = mybir.dt.float32
F32R = mybir.dt.float32r
BF16 = mybir.dt.bfloat16
AF = mybir.ActivationFunctionType
ALU = mybir.AluOpType

B = 8
C = 512
INTER = 128
BN_EPS = 1e-5


def _r(ap):
    """fp32 AP -> float32r view for full-rate PE matmuls."""
    return ap.bitcast(F32R)


def build_nc(S=64, n_cores=8):
    """Build + compile the per-core Bass program (SPMD, one sample/core)."""
    N = S * S
    GROWS = 2 * S - 1          # g table rows
    PADW = 2 * S               # padded g row width
    RPT = 128 // S             # grid rows per 128-row tile
    NT = N // 128              # n row-tiles
    MC = N // 128              # m chunks of 128
    NBLK = min(512, N)         # QK / rel free-dim block
    NMB = max(1, N // NBLK)    # m blocks per row-tile
    IMB = NBLK // S            # grid rows of m per block
    GW = GROWS * PADW + (RPT - 1) * PADW + S  # per-partition g buffer width
    UG = min(4, NT)            # row-tiles per u group (512 cols of n)
    TG = min(4, MC)            # rel-transpose group size
    OT = C // 128              # output-channel tiles
    KC = C // 128              # K chunks over C
    NH = 16 if N > 2048 else 1  # phase-1 n slices (SBUF pressure)
    NTOT = float(B * N)        # BN divisor

    nc = bacc.Bacc(
        "TRN2", target_bir_lowering=False, debug=False, num_devices=n_cores
    )

    x_in = nc.dram_tensor("x", [C, N], F32R, kind="ExternalInput").ap()
    wvt = nc.dram_tensor("wvt", [C, C], F32R, kind="ExternalInput").ap()
    wqt = nc.dram_tensor("wqt", [C, INTER], F32R, kind="ExternalInput").ap()
    wkt = nc.dram_tensor("wkt", [C, INTER], F32R, kind="ExternalInput").ap()
    wut = nc.dram_tensor("wut", [C, C], BF16, kind="ExternalInput").ap()
    gsrep_in = nc.dram_tensor("gsrep", [128, GW], F32, kind="ExternalInput").ap()
    ident_in = nc.dram_tensor("ident", [128, 128], BF16, kind="ExternalInput").ap()
    gamma_in = nc.dram_tensor("gamma", [128, OT], F32, kind="ExternalInput").ap()
    beta_in = nc.dram_tensor("beta", [128, OT], F32, kind="ExternalInput").ap()
    out_ext = nc.dram_tensor("out", [C, N], F32, kind="ExternalOutput").ap()

    with tile.TileContext(nc) as tc:
        with (
            tc.tile_pool(name="singles", bufs=1) as singles,
            tc.tile_pool(name="xc", bufs=KC + 2) as xpool,
            tc.tile_pool(name="work", bufs=3) as work,
            tc.tile_pool(name="uwork", bufs=2) as uwork,
            tc.tile_pool(name="relb", bufs=(N // min(512, N)) + 2) as relbp,
            tc.tile_pool(name="rT", bufs=3) as rTp,
            tc.tile_pool(name="yT", bufs=KC + 1) as yTp,
            tc.tile_pool(name="ysb", bufs=UG + 1) as ypool,
            tc.tile_pool(name="small", bufs=4) as small,
            tc.tile_pool(name="qk_ps", bufs=4, space="PSUM") as qkps,
            tc.tile_pool(name="tp_ps", bufs=2, space="PSUM") as tpps,
            tc.tile_pool(name="y_ps", bufs=2, space="PSUM") as yps,
            tc.tile_pool(name="dram", bufs=1, space="DRAM") as dram,
        ):
            # ---------- constants / weights ----------
            wvt_sb = singles.tile([128, KC, C], F32R, tag="wvt")
            nc.sync.dma_start(
                out=wvt_sb, in_=wvt.rearrange("(kc p) o -> p kc o", p=128)
            )
            wqt_sb = singles.tile([128, KC, INTER], F32R, tag="wqt")
            nc.sync.dma_start(
                out=wqt_sb, in_=wqt.rearrange("(kc p) i -> p kc i", p=128)
            )
            wkt_sb = singles.tile([128, KC, INTER], F32R, tag="wkt")
            nc.sync.dma_start(
                out=wkt_sb, in_=wkt.rearrange("(kc p) i -> p kc i", p=128)
            )
            wut_sb = singles.tile([128, KC, C], BF16, tag="wut")
            nc.sync.dma_start(
                out=wut_sb, in_=wut.rearrange("(kc p) o -> p kc o", p=128)
            )
            ident = singles.tile([128, 128], BF16, tag="ident")
            nc.sync.dma_start(out=ident, in_=ident_in)
            gamma_sb = singles.tile([128, OT], F32, tag="gamma")
            nc.sync.dma_start(out=gamma_sb, in_=gamma_in)
            beta_sb = singles.tile([128, OT], F32, tag="beta")
            nc.sync.dma_start(out=beta_sb, in_=beta_in)
            eps_sb = singles.tile([128, 1], F32, tag="eps")
            nc.vector.memset(eps_sb, BN_EPS)

            q_sb = singles.tile([128, N], F32R, tag="q")
            k_sb = singles.tile([128, N], F32R, tag="k")
            v_sb = singles.tile([128, MC, 512], BF16, tag="v")
            gsrep = singles.tile([128, GW], F32, tag="gsrep")

            sum_acc = singles.tile([128, OT], F32, tag="sum_acc")
            ssq_acc = singles.tile([128, OT], F32, tag="ssq_acc")
            nc.vector.memset(sum_acc, 0.0)
            nc.vector.memset(ssq_acc, 0.0)

            u_dram = dram.tile([C, N], F32)

            # ---------- phase 1: projections (q, k, v) ----------
            NHW = N // NH
            for nh in range(NH):
                nlo = nh * NHW
                x_sb = [
                    xpool.tile([128, NHW], F32R, tag="xc", name=f"xc{nh}_{i}")
                    for i in range(KC)
                ]
                for cc in range(KC):
                    nc.sync.dma_start(
                        out=x_sb[cc],
                        in_=x_in[cc * 128 : (cc + 1) * 128, nlo : nlo + NHW],
                    )
                PB = min(512, NHW)
                for nb in range(NHW // PB):
                    ps_q = qkps.tile([128, 512], F32, tag="qk")
                    ps_k = qkps.tile([128, 512], F32, tag="qk")
                    for cc in range(KC):
                        nc.tensor.matmul(
                            ps_q[:INTER, :PB],
                            wqt_sb[:, cc, :],
                            x_sb[cc][:, nb * PB : (nb + 1) * PB],
                            start=(cc == 0),
                            stop=(cc == KC - 1),
                        )
                    for cc in range(KC):
                        nc.tensor.matmul(
                            ps_k[:INTER, :PB],
                            wkt_sb[:, cc, :],
                            x_sb[cc][:, nb * PB : (nb + 1) * PB],
                            start=(cc == 0),
                            stop=(cc == KC - 1),
                        )
                    col = nlo + nb * PB
                    nc.vector.tensor_copy(
                        q_sb[:INTER, col : col + PB], ps_q[:INTER, :PB]
                    )
                    nc.vector.tensor_copy(
                        k_sb[:INTER, col : col + PB], ps_k[:INTER, :PB]
                    )
                for mt in range(NHW // 128):
                    mtg = nlo // 128 + mt
                    ps_v = qkps.tile([128, 512], F32, tag="qk")
                    for cc in range(KC):
                        nc.tensor.matmul(
                            ps_v,
                            x_sb[cc][:, mt * 128 : (mt + 1) * 128],
                            wvt_sb[:, cc, :],
                            start=(cc == 0),
                            stop=(cc == KC - 1),
                        )
                    nc.vector.tensor_copy(v_sb[:, mtg, :], ps_v)

            nc.sync.dma_start(out=gsrep, in_=gsrep_in)

            # ---------- phase 2: attention + u ----------
            QKB = min(512, N)      # softmax block width (1 PSUM bank)
            NQB = N // QKB         # blocks per row-tile
            IM2 = QKB // S         # grid rows of m per block
            y_list = []
            for t in range(NT):
                i0 = t * RPT
                # per-partition window offset into the shifted g table
                off = (S - 1 - i0) * PADW + (S - 1)
                relb = []
                den8 = small.tile([128, NQB], F32, tag="den8")
                for mb in range(NQB):
                    ps = qkps.tile([128, QKB], F32, tag="qk")
                    for hf in range(QKB // NBLK):
                        nc.tensor.matmul(
                            ps[:, hf * NBLK : (hf + 1) * NBLK],
                            q_sb[:INTER, t * 128 : (t + 1) * 128],
                            k_sb[
                                :INTER,
                                mb * QKB + hf * NBLK : mb * QKB + (hf + 1) * NBLK,
                            ],
                            start=True,
                            stop=True,
                        )
                    gwin = gsrep[
                        :, off + mb * IM2 * PADW : off + (mb + 1) * IM2 * PADW
                    ].rearrange("p (i j) -> p i j", j=PADW)[:, :, :S]
                    rf = work.tile([128, QKB], F32, tag="relf", bufs=2)
                    nc.vector.tensor_mul(
                        rf.rearrange("p (i j) -> p i j", j=S),
                        ps.rearrange("p (i j) -> p i j", j=S),
                        gwin,
                    )
                    rb = relbp.tile([128, QKB], BF16, tag="relb")
                    nc.scalar.activation(
                        out=rb,
                        in_=rf,
                        func=AF.Exp,
                        accum_out=den8[:, mb : mb + 1],
                    )
                    relb.append(rb)
                den = small.tile([128, 1], F32, tag="den")
                nc.vector.reduce_sum(den, den8, axis=mybir.AxisListType.X)
                rden = small.tile([128, 1], F32, tag="rden")
                nc.vector.reciprocal(rden, den)

                y_ps = yps.tile([128, 512], F32, tag="y")
                for tg in range(MC // TG):
                    tp = tpps.tile([128, 512], BF16, tag="tp")
                    for j in range(TG):
                        mc = tg * TG + j
                        blk = relb[mc * 128 // QKB]
                        co = (mc * 128) % QKB
                        nc.tensor.transpose(
                            tp[:, j * 128 : (j + 1) * 128],
                            blk[:, co : co + 128],
                            ident,
                        )
                    rT = rTp.tile([128, 512], BF16, tag="rT")
                    nc.vector.tensor_copy(
                        rT[:, : TG * 128], tp[:, : TG * 128]
                    )
                    for j in range(TG):
                        mc = tg * TG + j
                        nc.tensor.matmul(
                            y_ps,
                            rT[:, j * 128 : (j + 1) * 128],
                            v_sb[:, mc, :],
                            start=(mc == 0),
                            stop=(mc == MC - 1),
                        )
                y_sb = ypool.tile([128, 512], BF16, tag="y_sb")
                nc.vector.tensor_scalar(
                    out=y_sb, in0=y_ps, scalar1=rden, scalar2=None, op0=ALU.mult
                )
                y_list.append(y_sb)

                # ---------- u for each completed group of UG row-tiles ----------
                if t % UG == UG - 1:
                    g4 = t // UG
                    ys = y_list[g4 * UG : (g4 + 1) * UG]
                    un = UG * 128
                    yT = []
                    for c in range(OT):
                        tp2 = tpps.tile([128, 512], BF16, tag="tp")
                        for r2 in range(UG):
                            nc.tensor.transpose(
                                tp2[:, r2 * 128 : (r2 + 1) * 128],
                                ys[r2][:, c * 128 : (c + 1) * 128],
                                ident,
                            )
                        yTc = yTp.tile([128, 512], BF16, tag="yT")
                        nc.vector.tensor_copy(yTc[:, :un], tp2[:, :un])
                        yT.append(yTc)
                    for ot in range(OT):
                        u_ps = yps.tile([128, 512], F32, tag="y")
                        for cc2 in range(KC):
                            nc.tensor.matmul(
                                u_ps[:, :un],
                                wut_sb[:, cc2, ot * 128 : (ot + 1) * 128],
                                yT[cc2][:, :un],
                                start=(cc2 == 0),
                                stop=(cc2 == KC - 1),
                            )
                        u_sb = uwork.tile([128, 512], F32, tag="usb")
                        su = small.tile([128, 1], F32, tag="su")
                        nc.vector.tensor_scalar(
                            out=u_sb[:, :un],
                            in0=u_ps[:, :un],
                            scalar1=1.0,
                            scalar2=0.0,
                            op0=ALU.mult,
                            op1=ALU.add,
                            accum_out=su,
                        )
                        usq = uwork.tile([128, 512], F32, tag="usq", bufs=1)
                        ssq = small.tile([128, 1], F32, tag="ssq")
                        nc.scalar.activation(
                            out=usq[:, :un],
                            in_=u_ps[:, :un],
                            func=AF.Square,
                            accum_out=ssq,
                        )
                        nc.vector.tensor_add(
                            sum_acc[:, ot : ot + 1], sum_acc[:, ot : ot + 1], su
                        )
                        nc.vector.tensor_add(
                            ssq_acc[:, ot : ot + 1], ssq_acc[:, ot : ot + 1], ssq
                        )
                        nc.sync.dma_start(
                            out=u_dram[
                                ot * 128 : (ot + 1) * 128, g4 * un : (g4 + 1) * un
                            ],
                            in_=u_sb[:, :un],
                        )

            # ---------- phase 3: BN stats AllReduce ----------
            stats_sb = small.tile([128, 2 * OT], F32, tag="stats")
            nc.vector.tensor_copy(stats_sb[:, :OT], sum_acc)
            nc.vector.tensor_copy(stats_sb[:, OT:], ssq_acc)
            cc_in = dram.tile([128, 2 * OT], F32)
            cc_out = dram.tile([128, 2 * OT], F32)
            nc.sync.dma_start(out=cc_in, in_=stats_sb)
            nc.gpsimd.collective_compute(
                "AllReduce",
                ALU.add,
                replica_groups=[list(range(n_cores))],
                ins=[cc_in.opt()],
                outs=[cc_out.opt()],
            )
            gstats = small.tile([128, 2 * OT], F32, tag="gstats")
            nc.sync.dma_start(out=gstats, in_=cc_out)

            mean = small.tile([128, OT], F32, tag="mean")
            nc.vector.tensor_scalar(
                out=mean, in0=gstats[:, :OT], scalar1=1.0 / NTOT,
                scalar2=None, op0=ALU.mult,
            )
            ex2 = small.tile([128, OT], F32, tag="ex2")
            nc.vector.tensor_scalar(
                out=ex2, in0=gstats[:, OT:], scalar1=1.0 / NTOT,
                scalar2=None, op0=ALU.mult,
            )
            var = small.tile([128, OT], F32, tag="var")
            nc.vector.tensor_mul(var, mean, mean)
            nc.vector.tensor_sub(var, ex2, var)
            std = small.tile([128, OT], F32, tag="std")
            nc.scalar.activation(out=std, in_=var, func=AF.Sqrt, bias=eps_sb)
            rstd = small.tile([128, OT], F32, tag="rstd")
            nc.vector.reciprocal(rstd, std)
            scale = small.tile([128, OT], F32, tag="scale")
            nc.vector.tensor_mul(scale, gamma_sb, rstd)
            shift = small.tile([128, OT], F32, tag="shift")
            nc.vector.tensor_mul(shift, mean, scale)
            nc.vector.tensor_sub(shift, beta_sb, shift)

            # ---------- phase 4: normalize + residual + relu ----------
            FW = min(256, N)
            for ot in range(OT):
                for h in range(N // FW):
                    ut = work.tile([128, FW], F32, tag="f_u", bufs=4)
                    xt = work.tile([128, FW], F32, tag="f_x", bufs=4)
                    nc.sync.dma_start(
                        out=ut,
                        in_=u_dram[
                            ot * 128 : (ot + 1) * 128, h * FW : (h + 1) * FW
                        ],
                    )
                    nc.sync.dma_start(
                        out=xt,
                        in_=x_in[
                            ot * 128 : (ot + 1) * 128, h * FW : (h + 1) * FW
                        ].bitcast(F32),
                    )
                    nc.vector.tensor_scalar(
                        out=ut,
                        in0=ut,
                        scalar1=scale[:, ot : ot + 1],
                        scalar2=shift[:, ot : ot + 1],
                        op0=ALU.mult,
                        op1=ALU.add,
                    )
                    nc.vector.tensor_add(ut, ut, xt)
                    o_sb = work.tile([128, FW], F32, tag="f_o", bufs=4)
                    nc.scalar.activation(out=o_sb, in_=ut, func=AF.Relu)
                    nc.sync.dma_start(
                        out=out_ext[
                            ot * 128 : (ot + 1) * 128, h * FW : (h + 1) * FW
                        ],
                        in_=o_sb,
                    )

    nc.compile()
    return nc


def make_host_inputs(x, Wv, Wq, Wk, Wu, bn_gamma, bn_beta, S=64):
    """Prepare per-core input maps (host-side constant prep)."""
    N = S * S
    GROWS = 2 * S - 1
    PADW = 2 * S
    RPT = 128 // S
    GW = GROWS * PADW + (RPT - 1) * PADW + S
    OT = C // 128

    # 1/dist sliding-window table: g[p, q] = 1/(hypot(p-(S-1), q-(S-1))+1).
    # g is symmetric under axis reversal, so the same table serves all rows.
    dd = np.arange(GROWS, dtype=np.float32) - (S - 1)
    g = 1.0 / (np.sqrt(dd[:, None] ** 2 + dd[None, :] ** 2) + 1.0)
    gpad = np.zeros((GROWS, PADW), dtype=np.float32)
    gpad[:, :GROWS] = g
    gflat = gpad.reshape(-1)
    gsrep = np.zeros((128, GW), dtype=np.float32)
    for r in range(128):
        off = (r // S) * PADW + (r % S)
        gsrep[r, off : off + gflat.size] = gflat

    wvt = np.ascontiguousarray(np.asarray(Wv).T).astype(np.float32)
    wqt = np.ascontiguousarray(np.asarray(Wq).T).astype(np.float32)
    wkt = np.ascontiguousarray(np.asarray(Wk).T).astype(np.float32)
    wut = np.ascontiguousarray(np.asarray(Wu).T).astype(ml_dtypes.bfloat16)
    ident = np.eye(128, dtype=ml_dtypes.bfloat16)
    gam = np.ascontiguousarray(
        np.asarray(bn_gamma).astype(np.float32).reshape(OT, 128).T
    )
    bet = np.ascontiguousarray(
        np.asarray(bn_beta).astype(np.float32).reshape(OT, 128).T
    )

    shared = dict(
        wvt=wvt, wqt=wqt, wkt=wkt, wut=wut,
        gsrep=gsrep, ident=ident, gamma=gam, beta=bet,
    )
    in_maps = []
    for b in range(x.shape[0]):
        m = dict(shared)
        m["x"] = np.ascontiguousarray(
            np.asarray(x[b]).reshape(C, N)
        ).astype(np.float32)
        in_maps.append(m)
    return in_maps


def _ensure_profile_hook():
    """This image's antenv lacks axon_hooks; synthesize it and register
    the ctypes NTFF hook so run_bass_kernel_spmd(trace=True) works."""
    import types

    try:
        import antenv.axon_hooks  # noqa: F401

        return
    except ImportError:
        pass
    try:
        import antenv
    except ImportError:
        return
    mod = types.ModuleType("antenv.axon_hooks")
    holder = {"h": None}
    mod.set_axon_ntff_profile_hook = lambda h: holder.__setitem__("h", h)
    mod.get_axon_ntff_profile_hook = lambda: holder["h"]
    sys.modules["antenv.axon_hooks"] = mod
    antenv.axon_hooks = mod
    try:
        from trn_agent_boot.trn_boot import _ntff_profile_via_ctypes

        hook = _ntff_profile_via_ctypes("/opt/axon/libaxon_pjrt.so")
        if hook is not None:
            mod.set_axon_ntff_profile_hook(hook)
    except Exception:
        pass


_NC_CACHE = {}


def kernel(x, Wv, Wq, Wk, Wu, bn_gamma, bn_beta):
    x = np.asarray(x)
    S = x.shape[2]
    n_cores = x.shape[0]
    key = (S, n_cores)
    if key not in _NC_CACHE:
        _NC_CACHE[key] = build_nc(S=S, n_cores=n_cores)
    nc = _NC_CACHE[key]

    in_maps = make_host_inputs(x, Wv, Wq, Wk, Wu, bn_gamma, bn_beta, S=S)
    trace = bool(int(os.environ.get("KERNEL_TRACE", "0")))
    if trace:
        _ensure_profile_hook()
    res = run_bass_kernel_spmd(
        nc, in_maps, core_ids=list(range(n_cores)), trace=trace
    )
    if trace and res.exec_time_ns is not None:
        print(f"HW exec time: {res.exec_time_ns} ns")
        kernel.last_exec_time_ns = res.exec_time_ns
    out = np.stack([r["out"].reshape(C, S, S) for r in res.results])
    return out.astype(np.float32)
